# revision 1
# baseline (speedup 1.0000x reference)
"""DiffPool pooling layer on 8 Trainium2 NeuronCores.

Reference computation (edge_index / batch are unused by the output):
    s      = softmax(x @ Wp + bp, axis=-1)        # [N, C]
    h      = x @ We + be                          # [N, F]
    pooled = s^T @ h                              # [C, F]
    out    = pooled[None] @ Wo + bo               # [1, C, O]

Algebraic restructuring (projection is linear):
    pooled = (s^T x) We + colsum(s) be^T
so per node-shard k each core computes the partials
    G_k  = s_k^T x_k            [C, F]
    cs_k = colsum(s_k)          [C]
    out_k = (G_k We + cs_k be^T) Wo + bo/8
and the host sums the eight [C, O] partials (the unshard step).
No h materialization, no collectives.

Layout: nodes are block-assigned to partitions (partition p holds nodes
p*48..p*48+47 of the first 6144; the 106-node tail is node-major). This
makes the x DMA 16KB-contiguous per partition line (descriptor-cheap).
Any node->partition assignment is valid because the G contraction only
requires s and x to agree on it.

Per 128-node tile j (x resident in SBUF as fp16, cast during SWDGE DMA):
  - PE transposes 4 f-chunks -> xT (fp16 PSUM) -> DVE copy to SBUF
  - logits = ones x bp + sum_k xT_k^T @ Wp_k    (fp16 MMs, fp32 PSUM)
  - ACT exp w/ accum_out -> unnormalized s + row sums; DVE recip+scale
  - G/cs matmuls are software-pipelined several tiles behind so the PE
    does not stall on the softmax chain.
Final (once per core): project the partial in fp32 on PE.
Measured ~66-70us per core-pass on HW (8 cores in parallel); the x load
(12.8MB fp32 -> fp16 cast-DMA) is ~43us of that and overlaps compute.
"""

import numpy as np
from contextlib import ExitStack

N_ALL, F, C, O = 50000, 512, 64, 256
NCORES = 8
NLOC = N_ALL // NCORES          # 6250 nodes per core
P = 128
KC = F // P                     # 4 feature chunks
JROWS = 48                      # node tiles in the main block
NMAIN = P * JROWS               # 6144 nodes in the main block
NTAIL = NLOC - NMAIN            # 106-node tail
NSPLIT = 12                     # main-block DMA slices (4 tiles each)
JS = JROWS // NSPLIT            # tiles per slice

_CACHE = {}


def _main_loop(nc, mybir, x_d, xs_parts, x_tail,
               xtpool, spool, smallp, pxt, plg,
               ident16, ones_row16, ones_col16, wp_h, bp_h, g_ps, cs_ps,
               parts="full"):
    """One full pass over this core's node shard, accumulating G / colsum."""
    f32 = mybir.dt.float32
    f16 = mybir.dt.float16
    AF = mybir.ActivationFunctionType

    # x DMAs: main block as NSPLIT slices, 16KB contiguous per partition
    xm = x_d[0:NMAIN, :].rearrange("(p j) f -> p j f", p=P)
    if parts == "dma32":
        # ablation: HWDGE fp32 loads (no cast) into fp32 scratch
        for i in range(NSPLIT):
            nc.sync.dma_start(xs_parts[i][:], xm[:, i * JS : (i + 1) * JS, :])
        nc.sync.dma_start(x_tail[0:NTAIL, :], x_d[NMAIN:NLOC, :])
        return
    nc.gpsimd.dma_start(x_tail[0:NTAIL, :], x_d[NMAIN:NLOC, :])
    for i in range(NSPLIT):
        nc.gpsimd.dma_start(xs_parts[i][:], xm[:, i * JS : (i + 1) * JS, :])

    if parts == "dma":
        return

    # tile list: (x view full-partition, active rows)
    tiles = [(xs_parts[j // JS][:, j % JS, :], P) for j in range(JROWS)]
    tiles.append((x_tail[:, :], NTAIL))
    ntiles = len(tiles)

    # software pipeline so PE never waits on DVE/ACT:
    # at step j, PE runs: transp(j+1) | logits(j) | G/cs(j-SKEW)
    xt_sbs = {}   # j -> xt_sb
    s_views = {}  # j -> s view for G/cs

    def emit_transp(j):
        xv, nt = tiles[j]
        xt_ps = pxt.tile([P, KC, P], f16, tag="xt_ps", name="xt_ps")
        for k in range(KC):
            nc.tensor.transpose(
                xt_ps[:, k, 0:nt],
                xv[0:nt, k * P : (k + 1) * P],
                ident16[0:nt, 0:nt],
            )
        xt_sb = xtpool.tile([P, KC, P], f16, tag="xt_sb", name="xt_sb")
        # fp16 pairs copied as fp32 halves the DVE element count
        nc.vector.tensor_copy(
            xt_sb[:, :, 0:nt].bitcast(f32), xt_ps[:, :, 0:nt].bitcast(f32)
        )
        xt_sbs[j] = xt_sb

    def emit_logits(j):
        _, nt = tiles[j]
        xt_sb = xt_sbs.pop(j)
        lg_ps = plg.tile([P, C], f32, tag="lg_ps", name="lg_ps")
        nc.tensor.matmul(
            lg_ps[0:nt, :], ones_row16[:, 0:nt], bp_h[:],
            start=True, stop=False,
        )
        for k in range(KC):
            nc.tensor.matmul(
                lg_ps[0:nt, :], xt_sb[:, k, 0:nt], wp_h[:, k, :],
                start=False, stop=(k == KC - 1),
            )
        return lg_ps

    def emit_softmax(j, lg_ps):
        _, nt = tiles[j]
        if parts == "nosm":
            s_views[j] = ident16[0:nt, 0:C]
            return
        se = spool.tile([P, C], f32, tag="se", name="se")
        rs = smallp.tile([P, 1], f32, tag="rs", name="rs")
        nc.scalar.activation(
            se[0:nt, :], lg_ps[0:nt, :], AF.Exp, accum_out=rs[0:nt, :]
        )
        ri = smallp.tile([P, 1], f32, tag="ri", name="ri")
        nc.vector.reciprocal(ri[0:nt, :], rs[0:nt, :])
        s_h = spool.tile([P, C], f16, tag="s_h", name="s_h")
        nc.vector.tensor_scalar_mul(s_h[0:nt, :], se[0:nt, :], ri[0:nt, :])
        s_views[j] = s_h[0:nt, :]

    def emit_gcs(j, last):
        xv, nt = tiles[j]
        s_view = s_views.pop(j)
        nc.tensor.matmul(
            g_ps[:], s_view, xv[0:nt, :],
            start=(j == 0), stop=last,
        )
        nc.tensor.matmul(
            cs_ps[:], ones_col16[0:nt, :], s_view,
            start=(j == 0), stop=last,
        )

    SKEW = 4
    emit_transp(0)
    for j in range(ntiles):
        if j + 1 < ntiles:
            emit_transp(j + 1)
        lg_ps = emit_logits(j)
        if j >= SKEW:
            emit_gcs(j - SKEW, last=False)
        emit_softmax(j, lg_ps)
    for j in range(ntiles - SKEW, ntiles):
        emit_gcs(j, last=(j == ntiles - 1))


def _build(bench_reps=None, parts="full"):
    """Build the bass module. bench_reps: if set, wrap the main node loop
    in a hardware For_i repeating it that many times (timing-only variant:
    x and weights live on device, no input transfer)."""
    import concourse.mybir as mybir
    import concourse.tile as tile
    from concourse import bacc
    from concourse.masks import make_identity

    f32 = mybir.dt.float32
    f16 = mybir.dt.float16

    nc = bacc.Bacc(
        "TRN2", target_bir_lowering=False, debug=False, num_devices=NCORES
    )

    if bench_reps:
        x_d = nc.dram_tensor("xint", [NLOC, F], f32, kind="Internal").ap()
        wp_d = bp_d = we_d = be_d = wo_d = bo_d = None
    else:
        x_d = nc.dram_tensor("x", [NLOC, F], f32, kind="ExternalInput").ap()
        wp_d = nc.dram_tensor("wp", [F, C], f32, kind="ExternalInput").ap()
        bp_d = nc.dram_tensor("bp", [1, C], f32, kind="ExternalInput").ap()
        we_d = nc.dram_tensor("we", [F, F], f32, kind="ExternalInput").ap()
        be_d = nc.dram_tensor("be", [1, F], f32, kind="ExternalInput").ap()
        wo_d = nc.dram_tensor("wo", [F, O], f32, kind="ExternalInput").ap()
        bo_d = nc.dram_tensor("bo8", [1, O], f32, kind="ExternalInput").ap()
    out_d = nc.dram_tensor("out", [C, O], f32, kind="ExternalOutput").ap()

    with tile.TileContext(nc) as tc, ExitStack() as ctx:
        const = ctx.enter_context(tc.tile_pool(name="const", bufs=1))
        accp = ctx.enter_context(tc.tile_pool(name="accp", bufs=1, space="PSUM"))

        ident16 = const.tile([P, P], f16)
        make_identity(nc, ident16[:])
        ident32 = const.tile([C, C], f32)
        make_identity(nc, ident32[:])
        ones_row16 = const.tile([1, P], f16)
        nc.vector.memset(ones_row16[:], 1.0)
        ones_col16 = const.tile([P, 1], f16)
        nc.vector.memset(ones_col16[:], 1.0)
        ones_row32 = const.tile([1, P], f32)
        nc.vector.memset(ones_row32[:], 1.0)

        # resident x (fp16): NSPLIT main slices + node-major tail
        xdt = f32 if parts == "dma32" else f16
        xs_parts = [
            const.tile([P, JS, F], xdt, name=f"xs{i}") for i in range(NSPLIT)
        ]
        x_tail = const.tile([P, F], xdt, name="x_tail")

        # weights: [F, M] -> [128, KC, M] (partition = f within chunk)
        wp_sb = const.tile([P, KC, C], f32)
        wp_h = const.tile([P, KC, C], f16)
        bp_h = const.tile([1, C], f16)
        we_sb = const.tile([P, KC, F], f32)
        be_sb = const.tile([1, F], f32)
        wo_sb = const.tile([P, KC, O], f32)
        bo_sb = const.tile([1, O], f32)
        if bench_reps:
            for tl in (wp_sb, bp_h, we_sb, be_sb, wo_sb, bo_sb):
                nc.vector.memset(tl[:], 0.0)
        else:
            nc.gpsimd.dma_start(
                wp_sb[:], wp_d.rearrange("(kc p) c -> p kc c", p=P)
            )
            nc.gpsimd.dma_start(bp_h[:], bp_d)  # cast during DMA
            nc.gpsimd.dma_start(
                we_sb[:], we_d.rearrange("(kc p) f -> p kc f", p=P)
            )
            nc.gpsimd.dma_start(be_sb[:], be_d)
            nc.gpsimd.dma_start(
                wo_sb[:], wo_d.rearrange("(kc p) o -> p kc o", p=P)
            )
            nc.gpsimd.dma_start(bo_sb[:], bo_d)
        nc.gpsimd.tensor_copy(wp_h[:], wp_sb[:])

        # persistent accumulators (one PSUM bank each)
        g_ps = accp.tile([C, F], f32)
        cs_ps = accp.tile([1, C], f32)

        if bench_reps:
            # zero-fill internal x so the compute sees finite data
            zt = const.tile([P, JS, F], f32, name="zt")
            nc.vector.memset(zt[:], 0.0)
            xm = x_d[0:NMAIN, :].rearrange("(p j) f -> p j f", p=P)
            for i in range(NSPLIT):
                nc.sync.dma_start(xm[:, i * JS : (i + 1) * JS, :], zt[:])
            nc.sync.dma_start(x_d[NMAIN:NLOC, :], zt[0:NTAIL, 0, :])

        with ExitStack() as lctx:
            xtpool = lctx.enter_context(tc.tile_pool(name="xtpool", bufs=4))
            spool = lctx.enter_context(tc.tile_pool(name="spool", bufs=8))
            smallp = lctx.enter_context(tc.tile_pool(name="smallp", bufs=8))
            pxt = lctx.enter_context(
                tc.tile_pool(name="pxt", bufs=3, space="PSUM")
            )
            plg = lctx.enter_context(
                tc.tile_pool(name="plg", bufs=3, space="PSUM")
            )

            rep_ctx = (
                tc.For_i(0, bench_reps, 1) if bench_reps else ExitStack()
            )
            with rep_ctx:
                _main_loop(
                    nc, mybir, x_d, xs_parts, x_tail,
                    xtpool, spool, smallp, pxt, plg,
                    ident16, ones_row16, ones_col16, wp_h, bp_h,
                    g_ps, cs_ps, parts=parts,
                )

        if parts in ("dma", "dma32"):
            with ExitStack() as fctx:
                fin0 = fctx.enter_context(tc.tile_pool(name="fin0", bufs=1))
                dummy = fin0.tile([C, O], f32, name="dummy")
                nc.vector.memset(dummy[:], 0.0)
                nc.sync.dma_start(out_d, dummy[:])
        elif True:
            # ---- final projection of the per-core partial (fp32) ----
            with ExitStack() as fctx:
                fin = fctx.enter_context(tc.tile_pool(name="fin", bufs=1))
                pfin = fctx.enter_context(
                    tc.tile_pool(name="pfin", bufs=1, space="PSUM")
                )

                g_sb = fin.tile([C, F], f32)
                nc.vector.tensor_copy(g_sb[:], g_ps[:])
                cs_sb = fin.tile([1, C], f32)
                nc.vector.tensor_copy(cs_sb[:], cs_ps[:])

                # G^T chunks [128, C] so fin lands on partitions
                gt_ps = pfin.tile([P, KC, C], f32, name="gt_ps")
                for k in range(KC):
                    nc.tensor.transpose(
                        gt_ps[:, k, :], g_sb[:, k * P : (k + 1) * P], ident32[:]
                    )
                gt_sb = fin.tile([P, KC, C], f32)
                nc.vector.tensor_copy(gt_sb[:], gt_ps[:])

                # pooledT[fo, c] = sum_fin We[fin, fo] G^T[fin, c] + be[fo] cs[c]
                pt_ps = pfin.tile([P, KC, C], f32, name="pt_ps")
                for j in range(KC):
                    nc.tensor.matmul(
                        pt_ps[:, j, :],
                        be_sb[:, j * P : (j + 1) * P],
                        cs_sb[:],
                        start=True,
                        stop=False,
                    )
                    for k in range(KC):
                        nc.tensor.matmul(
                            pt_ps[:, j, :],
                            we_sb[:, k, j * P : (j + 1) * P],
                            gt_sb[:, k, :],
                            start=False,
                            stop=(k == KC - 1),
                        )
                pt_sb = fin.tile([P, KC, C], f32)
                nc.vector.tensor_copy(pt_sb[:], pt_ps[:])

                # out[c, o] = sum_fo pooledT[fo, c] Wo[fo, o] + bo/8
                out_ps = pfin.tile([C, O], f32, name="out_ps")
                nc.tensor.matmul(
                    out_ps[:], ones_row32[:, 0:C], bo_sb[:],
                    start=True, stop=False,
                )
                for j in range(KC):
                    nc.tensor.matmul(
                        out_ps[:], pt_sb[:, j, :], wo_sb[:, j, :],
                        start=False, stop=(j == KC - 1),
                    )
                out_sb = fin.tile([C, O], f32)
                nc.vector.tensor_copy(out_sb[:], out_ps[:])
                nc.sync.dma_start(out_d, out_sb[:])

    nc.compile()
    return nc


def _get_nc(bench_reps=None, parts="full"):
    key = ("nc", bench_reps, parts)
    if key not in _CACHE:
        _CACHE[key] = _build(bench_reps, parts)
    return _CACHE[key]


def kernel(x, edge_index=None, batch=None, Wp=None, bp=None, We=None,
           be=None, Wo=None, bo=None, **_unused):
    from concourse.bass_utils import run_bass_kernel_spmd

    x = np.ascontiguousarray(np.asarray(x, dtype=np.float32))
    Wp = np.ascontiguousarray(np.asarray(Wp, dtype=np.float32))
    bp = np.ascontiguousarray(np.asarray(bp, dtype=np.float32)).reshape(1, C)
    We = np.ascontiguousarray(np.asarray(We, dtype=np.float32))
    be = np.ascontiguousarray(np.asarray(be, dtype=np.float32)).reshape(1, F)
    Wo = np.ascontiguousarray(np.asarray(Wo, dtype=np.float32))
    bo8 = np.ascontiguousarray(
        np.asarray(bo, dtype=np.float32).reshape(1, O) / np.float32(NCORES)
    )

    nc = _get_nc()
    in_maps = []
    for k in range(NCORES):
        in_maps.append(
            {
                "x": np.ascontiguousarray(x[k * NLOC : (k + 1) * NLOC]),
                "wp": Wp,
                "bp": bp,
                "we": We,
                "be": be,
                "wo": Wo,
                "bo8": bo8,
            }
        )
    res = run_bass_kernel_spmd(nc, in_maps, core_ids=list(range(NCORES)))
    out = np.zeros((C, O), np.float32)
    for r in res.results:
        out = out + r["out"]
    return out[None]  # [1, C, O]



# revision 4
# speedup vs baseline: 1.5850x; 1.5850x over previous
"""DiffPool pooling layer on 8 Trainium2 NeuronCores.

Reference computation (edge_index / batch are unused by the output):
    s      = softmax(x @ Wp + bp, axis=-1)        # [N, C]
    h      = x @ We + be                          # [N, F]
    pooled = s^T @ h                              # [C, F]
    out    = pooled[None] @ Wo + bo               # [1, C, O]

Algebraic restructuring (projection is linear):
    pooled = (s^T x) We + colsum(s) be^T
so per node-shard k each core computes the partials
    G_k  = s_k^T x_k            [C, F]
    cs_k = colsum(s_k)          [C]
    out_k = (G_k We + cs_k be^T) Wo + bo/8
and the host sums the eight [C, O] partials (the unshard step).
No h materialization, no collectives.

v2 changes vs the 69.5us baseline (which was compute-bound: ablations
measured full=68.3us, dma-only=46.4, compute-only=59.2):
  - x is cast to fp16 on the HOST and stored fp16 in HBM: halves the x
    DMA (the kernel always computed in fp16 anyway; rel-err unchanged).
    Loads go over HWDGE (nc.sync) since no cast is needed.
  - DVE de-bottlenecking: the xT PSUM->SBUF copies are batched two
    tiles per instruction (one full PSUM bank), and the softmax
    reciprocals are batched RB=8 tiles per instruction; exp writes
    fp16 so the s-scale runs in 2x DVE mode.
  - G/cs matmuls skewed SKEW_G=12 tiles behind to tolerate the batched
    recip latency.

Layout: nodes are block-assigned to partitions (partition p holds nodes
p*48..p*48+47 of the first 6144; the 106-node tail is node-major). This
makes the x DMA 4KB-contiguous per partition line (descriptor-cheap).
Any node->partition assignment is valid because the G contraction only
requires s and x to agree on it.

Per 128-node tile j (x resident in SBUF as fp16):
  - PE transposes 4 f-chunks -> xT (fp16 PSUM, pair-buffered)
  - logits = ones x bp + sum_k xT_k^T @ Wp_k    (fp16 MMs, fp32 PSUM)
  - ACT exp w/ accum_out -> unnormalized s (fp16) + row sums
  - DVE batched recip + per-tile scale -> s (fp16)
  - G/cs matmuls pipelined SKEW_G tiles behind
Final (once per core): project the partial in fp32 on PE.
"""

import numpy as np
from contextlib import ExitStack

N_ALL, F, C, O = 50000, 512, 64, 256
NCORES = 8
NLOC = N_ALL // NCORES          # 6250 nodes per core
P = 128
KC = F // P                     # 4 feature chunks
JROWS = 48                      # node tiles in the main block
NMAIN = P * JROWS               # 6144 nodes in the main block
NTAIL = NLOC - NMAIN            # 106-node tail
NSPLIT = 12                     # main-block DMA slices (4 tiles each)
JS = JROWS // NSPLIT            # tiles per slice
RB = 8                          # recip batch (tiles)
SKEW_G = 12                     # G matmul pipeline skew (tiles)

_CACHE = {}


def _main_loop(nc, mybir, x_d, xs_parts, x_tail,
               xtpool, spool, smallp, pxt, plg,
               ident16, ones_row16, ones_col16, wp_h, bp_h, g_ps, cs_ps,
               parts="full"):
    """One full pass over this core's node shard, accumulating G / colsum."""
    f32 = mybir.dt.float32
    f16 = mybir.dt.float16
    AF = mybir.ActivationFunctionType

    # x DMAs: main block as NSPLIT slices, 4KB contiguous per partition
    xm = x_d[0:NMAIN, :].rearrange("(p j) f -> p j f", p=P)
    if parts != "nodma":
        nc.sync.dma_start(x_tail[0:NTAIL, :], x_d[NMAIN:NLOC, :])
        for i in range(NSPLIT):
            nc.sync.dma_start(xs_parts[i][:], xm[:, i * JS : (i + 1) * JS, :])

    if parts == "dma":
        return

    # tile list: (x view full-partition, active rows)
    tiles = [(xs_parts[j // JS][:, j % JS, :], P) for j in range(JROWS)]
    tiles.append((x_tail[:, :], NTAIL))
    ntiles = len(tiles)

    pair_ps = {}   # pair idx -> xt psum pair tile
    pair_sb = {}   # pair idx -> xt sbuf pair tile
    se_hs = {}     # t -> unnormalized exp (fp16)
    rs_bufs = {}   # batch start -> row-sum accum [P, RB]
    ri_bufs = {}   # batch start -> reciprocal [P, RB]
    s_hs = {}      # t -> normalized s (fp16)

    def emit_transp(j):
        xv, nt = tiles[j]
        p_idx, slot = j // 2, j % 2
        if slot == 0:
            pair_ps[p_idx] = pxt.tile([P, 2, KC, P], f16, tag="xt_ps",
                                      name="xt_ps")
        xt_ps = pair_ps[p_idx]
        for k in range(KC):
            nc.tensor.transpose(
                xt_ps[:, slot, k, 0:nt],
                xv[0:nt, k * P : (k + 1) * P],
                ident16[0:nt, 0:nt],
            )

    def emit_paircopy(p_idx):
        xt_ps = pair_ps.pop(p_idx)
        xt_sb = xtpool.tile([P, 2, KC, P], f16, tag="xt_sb", name="xt_sb")
        lo = 2 * p_idx
        nslots = min(2, ntiles - lo)
        # fp16 pairs copied as fp32 halves the DVE element count
        ncols = tiles[lo + nslots - 1][1] if nslots == 1 else P
        if nslots == 2 and tiles[lo + 1][1] != P:
            # mixed pair: copy full width of slot0, partial of slot1
            nc.vector.tensor_copy(
                xt_sb[:, 0, :, :].bitcast(f32), xt_ps[:, 0, :, :].bitcast(f32)
            )
            nt1 = tiles[lo + 1][1]
            nc.vector.tensor_copy(
                xt_sb[:, 1, :, 0:nt1].bitcast(f32),
                xt_ps[:, 1, :, 0:nt1].bitcast(f32),
            )
        else:
            nc.vector.tensor_copy(
                xt_sb[:, 0:nslots, :, 0:ncols].bitcast(f32),
                xt_ps[:, 0:nslots, :, 0:ncols].bitcast(f32),
            )
        pair_sb[p_idx] = xt_sb

    def emit_logits(t):
        _, nt = tiles[t]
        p_idx, slot = t // 2, t % 2
        xt_sb = pair_sb[p_idx]
        if slot == 1 or p_idx * 2 + 1 >= ntiles:
            del pair_sb[p_idx]
        lg_ps = plg.tile([P, C], f32, tag="lg_ps", name="lg_ps")
        nc.tensor.matmul(
            lg_ps[0:nt, :], ones_row16[:, 0:nt], bp_h[:],
            start=True, stop=False,
        )
        for k in range(KC):
            nc.tensor.matmul(
                lg_ps[0:nt, :], xt_sb[:, slot, k, 0:nt], wp_h[:, k, :],
                start=False, stop=(k == KC - 1),
            )
        return lg_ps

    def emit_exp(t, lg_ps):
        _, nt = tiles[t]
        if parts == "nosm":
            s_hs[t] = ident16[0:nt, 0:C]
            return
        b = t - (t % RB)
        if t == b:
            rs_bufs[b] = smallp.tile([P, RB], f32, tag="rs", name="rs")
        se = spool.tile([P, C], f16, tag="se", name="se")
        nc.scalar.activation(
            se[0:nt, :], lg_ps[0:nt, :], AF.Exp,
            accum_out=rs_bufs[b][0:nt, t - b : t - b + 1],
        )
        se_hs[t] = se

    def emit_recip_scales(t_last):
        if parts == "nosm":
            return
        b = t_last - (t_last % RB)
        w = t_last - b + 1
        rs = rs_bufs.pop(b)
        ri = smallp.tile([P, RB], f32, tag="ri", name="ri")
        nc.vector.reciprocal(ri[:, 0:w], rs[:, 0:w])
        ri_bufs[b] = ri
        for u in range(b, t_last + 1):
            _, nt = tiles[u]
            se = se_hs.pop(u)
            s_h = spool.tile([P, C], f16, tag="s_h", name="s_h")
            nc.vector.tensor_scalar_mul(
                s_h[0:nt, :], se[0:nt, :], ri[0:nt, u - b : u - b + 1]
            )
            s_hs[u] = s_h

    def emit_g(t, last):
        xv, nt = tiles[t]
        s_view = s_hs.pop(t)
        if parts != "nosm":
            s_view = s_view[0:nt, :]
        nc.tensor.matmul(
            g_ps[:], s_view, xv[0:nt, :],
            start=(t == 0), stop=last,
        )
        nc.tensor.matmul(
            cs_ps[:], ones_col16[0:nt, :], s_view,
            start=(t == 0), stop=last,
        )

    steps = ntiles + 2
    for j in range(steps):
        if j < ntiles:
            emit_transp(j)
            if j % 2 == 1:
                emit_paircopy(j // 2)
        if j == ntiles and ntiles % 2 == 1:
            emit_paircopy(ntiles // 2)
        t = j - 2
        if 0 <= t < ntiles:
            lg_ps = emit_logits(t)
            emit_exp(t, lg_ps)
            if t % RB == RB - 1 or t == ntiles - 1:
                emit_recip_scales(t)
        g = j - SKEW_G
        if 0 <= g < ntiles:
            emit_g(g, last=(g == ntiles - 1))
    for g in range(max(0, steps - SKEW_G), ntiles):
        emit_g(g, last=(g == ntiles - 1))


def _build(bench_reps=None, parts="full"):
    """Build the bass module. bench_reps: if set, wrap the main node loop
    in a hardware For_i repeating it that many times (timing-only variant:
    x and weights live on device, no input transfer)."""
    import concourse.mybir as mybir
    import concourse.tile as tile
    from concourse import bacc
    from concourse.masks import make_identity

    f32 = mybir.dt.float32
    f16 = mybir.dt.float16

    nc = bacc.Bacc(
        "TRN2", target_bir_lowering=False, debug=False, num_devices=NCORES
    )

    if bench_reps:
        x_d = nc.dram_tensor("xint", [NLOC, F], f16, kind="Internal").ap()
        wp_d = bp_d = we_d = be_d = wo_d = bo_d = None
    else:
        x_d = nc.dram_tensor("x16", [NLOC, F], f16, kind="ExternalInput").ap()
        wp_d = nc.dram_tensor("wp", [F, C], f32, kind="ExternalInput").ap()
        bp_d = nc.dram_tensor("bp", [1, C], f32, kind="ExternalInput").ap()
        we_d = nc.dram_tensor("we", [F, F], f32, kind="ExternalInput").ap()
        be_d = nc.dram_tensor("be", [1, F], f32, kind="ExternalInput").ap()
        wo_d = nc.dram_tensor("wo", [F, O], f32, kind="ExternalInput").ap()
        bo_d = nc.dram_tensor("bo8", [1, O], f32, kind="ExternalInput").ap()
    out_d = nc.dram_tensor("out", [C, O], f32, kind="ExternalOutput").ap()

    with tile.TileContext(nc) as tc, ExitStack() as ctx:
        const = ctx.enter_context(tc.tile_pool(name="const", bufs=1))
        accp = ctx.enter_context(tc.tile_pool(name="accp", bufs=1, space="PSUM"))

        ident16 = const.tile([P, P], f16)
        make_identity(nc, ident16[:])
        ident32 = const.tile([C, C], f32)
        make_identity(nc, ident32[:])
        ones_row16 = const.tile([1, P], f16)
        nc.vector.memset(ones_row16[:], 1.0)
        ones_col16 = const.tile([P, 1], f16)
        nc.vector.memset(ones_col16[:], 1.0)
        ones_row32 = const.tile([1, P], f32)
        nc.vector.memset(ones_row32[:], 1.0)

        # resident x (fp16): NSPLIT main slices + node-major tail
        xs_parts = [
            const.tile([P, JS, F], f16, name=f"xs{i}") for i in range(NSPLIT)
        ]
        x_tail = const.tile([P, F], f16, name="x_tail")

        # weights: [F, M] -> [128, KC, M] (partition = f within chunk)
        wp_sb = const.tile([P, KC, C], f32)
        wp_h = const.tile([P, KC, C], f16)
        bp_h = const.tile([1, C], f16)
        we_sb = const.tile([P, KC, F], f32)
        be_sb = const.tile([1, F], f32)
        wo_sb = const.tile([P, KC, O], f32)
        bo_sb = const.tile([1, O], f32)
        if bench_reps:
            for tl in (wp_sb, bp_h, we_sb, be_sb, wo_sb, bo_sb):
                nc.vector.memset(tl[:], 0.0)
        else:
            nc.gpsimd.dma_start(
                wp_sb[:], wp_d.rearrange("(kc p) c -> p kc c", p=P)
            )
            nc.gpsimd.dma_start(bp_h[:], bp_d)  # cast during DMA
            nc.gpsimd.dma_start(
                we_sb[:], we_d.rearrange("(kc p) f -> p kc f", p=P)
            )
            nc.gpsimd.dma_start(be_sb[:], be_d)
            nc.gpsimd.dma_start(
                wo_sb[:], wo_d.rearrange("(kc p) o -> p kc o", p=P)
            )
            nc.gpsimd.dma_start(bo_sb[:], bo_d)
        nc.gpsimd.tensor_copy(wp_h[:], wp_sb[:])

        # persistent accumulators (one PSUM bank each)
        g_ps = accp.tile([C, F], f32)
        cs_ps = accp.tile([1, C], f32)

        if bench_reps:
            # zero-fill internal x so the compute sees finite data
            zt = const.tile([P, JS, F], f16, name="zt")
            nc.vector.memset(zt[:], 0.0)
            xm = x_d[0:NMAIN, :].rearrange("(p j) f -> p j f", p=P)
            for i in range(NSPLIT):
                nc.sync.dma_start(xm[:, i * JS : (i + 1) * JS, :], zt[:])
            nc.sync.dma_start(x_d[NMAIN:NLOC, :], zt[0:NTAIL, 0, :])
        if parts == "nodma":
            for t in xs_parts:
                nc.vector.memset(t[:], 0.0)
            nc.vector.memset(x_tail[:], 0.0)

        with ExitStack() as lctx:
            xtpool = lctx.enter_context(tc.tile_pool(name="xtpool", bufs=4))
            spool = lctx.enter_context(tc.tile_pool(name="spool", bufs=24))
            smallp = lctx.enter_context(tc.tile_pool(name="smallp", bufs=4))
            pxt = lctx.enter_context(
                tc.tile_pool(name="pxt", bufs=3, space="PSUM")
            )
            plg = lctx.enter_context(
                tc.tile_pool(name="plg", bufs=3, space="PSUM")
            )

            rep_ctx = (
                tc.For_i(0, bench_reps, 1) if bench_reps else ExitStack()
            )
            with rep_ctx:
                _main_loop(
                    nc, mybir, x_d, xs_parts, x_tail,
                    xtpool, spool, smallp, pxt, plg,
                    ident16, ones_row16, ones_col16, wp_h, bp_h,
                    g_ps, cs_ps, parts=parts,
                )

        if parts == "dma":
            with ExitStack() as fctx:
                fin0 = fctx.enter_context(tc.tile_pool(name="fin0", bufs=1))
                dummy = fin0.tile([C, O], f32, name="dummy")
                nc.vector.memset(dummy[:], 0.0)
                nc.sync.dma_start(out_d, dummy[:])
        else:
            # ---- final projection of the per-core partial (fp32) ----
            with ExitStack() as fctx:
                fin = fctx.enter_context(tc.tile_pool(name="fin", bufs=1))
                pfin = fctx.enter_context(
                    tc.tile_pool(name="pfin", bufs=1, space="PSUM")
                )

                g_sb = fin.tile([C, F], f32)
                nc.vector.tensor_copy(g_sb[:], g_ps[:])
                cs_sb = fin.tile([1, C], f32)
                nc.vector.tensor_copy(cs_sb[:], cs_ps[:])

                # G^T chunks [128, C] so fin lands on partitions
                gt_ps = pfin.tile([P, KC, C], f32, name="gt_ps")
                for k in range(KC):
                    nc.tensor.transpose(
                        gt_ps[:, k, :], g_sb[:, k * P : (k + 1) * P], ident32[:]
                    )
                gt_sb = fin.tile([P, KC, C], f32)
                nc.vector.tensor_copy(gt_sb[:], gt_ps[:])

                # pooledT[fo, c] = sum_fin We[fin, fo] G^T[fin, c] + be[fo] cs[c]
                pt_ps = pfin.tile([P, KC, C], f32, name="pt_ps")
                for j in range(KC):
                    nc.tensor.matmul(
                        pt_ps[:, j, :],
                        be_sb[:, j * P : (j + 1) * P],
                        cs_sb[:],
                        start=True,
                        stop=False,
                    )
                    for k in range(KC):
                        nc.tensor.matmul(
                            pt_ps[:, j, :],
                            we_sb[:, k, j * P : (j + 1) * P],
                            gt_sb[:, k, :],
                            start=False,
                            stop=(k == KC - 1),
                        )
                pt_sb = fin.tile([P, KC, C], f32)
                nc.vector.tensor_copy(pt_sb[:], pt_ps[:])

                # out[c, o] = sum_fo pooledT[fo, c] Wo[fo, o] + bo/8
                out_ps = pfin.tile([C, O], f32, name="out_ps")
                nc.tensor.matmul(
                    out_ps[:], ones_row32[:, 0:C], bo_sb[:],
                    start=True, stop=False,
                )
                for j in range(KC):
                    nc.tensor.matmul(
                        out_ps[:], pt_sb[:, j, :], wo_sb[:, j, :],
                        start=False, stop=(j == KC - 1),
                    )
                out_sb = fin.tile([C, O], f32)
                nc.vector.tensor_copy(out_sb[:], out_ps[:])
                nc.sync.dma_start(out_d, out_sb[:])

    nc.compile()
    return nc


def _get_nc(bench_reps=None, parts="full"):
    key = ("nc", bench_reps, parts)
    if key not in _CACHE:
        _CACHE[key] = _build(bench_reps, parts)
    return _CACHE[key]


def kernel(x, edge_index=None, batch=None, Wp=None, bp=None, We=None,
           be=None, Wo=None, bo=None, **_unused):
    from concourse.bass_utils import run_bass_kernel_spmd

    x16 = np.ascontiguousarray(np.asarray(x, dtype=np.float32)).astype(
        np.float16
    )
    Wp = np.ascontiguousarray(np.asarray(Wp, dtype=np.float32))
    bp = np.ascontiguousarray(np.asarray(bp, dtype=np.float32)).reshape(1, C)
    We = np.ascontiguousarray(np.asarray(We, dtype=np.float32))
    be = np.ascontiguousarray(np.asarray(be, dtype=np.float32)).reshape(1, F)
    Wo = np.ascontiguousarray(np.asarray(Wo, dtype=np.float32))
    bo8 = np.ascontiguousarray(
        np.asarray(bo, dtype=np.float32).reshape(1, O) / np.float32(NCORES)
    )

    nc = _get_nc()
    in_maps = []
    for k in range(NCORES):
        in_maps.append(
            {
                "x16": np.ascontiguousarray(x16[k * NLOC : (k + 1) * NLOC]),
                "wp": Wp,
                "bp": bp,
                "we": We,
                "be": be,
                "wo": Wo,
                "bo8": bo8,
            }
        )
    res = run_bass_kernel_spmd(nc, in_maps, core_ids=list(range(NCORES)))
    out = np.zeros((C, O), np.float32)
    for r in res.results:
        out = out + r["out"]
    return out[None]  # [1, C, O]
